# revision 3
# baseline (speedup 1.0000x reference)
"""EvolveGCN (GRU-evolved GCN layer + linear head) on 8 Trainium2 NeuronCores.

Strategy: shard destination nodes (and their incoming edges) across cores.
Each core owns 49 tiles of 128 destination nodes. Edges (incl. self-loops)
are sorted by destination tile on host; per dest tile the kernel gathers
source rows of x with indirect DMA, builds a norm-scaled one-hot matrix on
DVE, and accumulates h_pre^T = sum_e x[src_e]^T (x) onehot(dst_e)*norm_e in
PSUM via TensorE. Then h^T = W^T-matmul (using (A X) W == A (X W)), relu,
and the [128,8] output head, all on device. The small GRU weight evolution
runs replicated on every core.
"""

import os
import sys

import numpy as np

try:
    import concourse.bass as bass
except ImportError:  # fresh grading dir: fall back to the staged repo
    for p in ("/opt/trn_rl_repo", "/root/.axon_site/_ro/trn_rl_repo"):
        if os.path.isdir(p) and p not in sys.path:
            sys.path.insert(0, p)
    import concourse.bass as bass

import concourse.mybir as mybir
import concourse.tile as tile
from concourse import bacc
from concourse.bass_utils import run_bass_kernel_spmd
from concourse.masks import make_identity

P = 128
N_NODES = 50000
F_DIM = 128
N_TARGET = 8
NCORES = 8
TILES_PER_CORE = 49            # 49 * 128 = 6272; 8 * 6272 = 50176 >= 50000
ROWS_PER_CORE = TILES_PER_CORE * P
NT_GLOBAL = NCORES * TILES_PER_CORE  # 392 padded global tiles

_COMPILED = {}


def _host_prep(x, edge_index, edge_weight):
    """Sort/pad edges by destination tile; returns per-core index/meta arrays."""
    row = np.asarray(edge_index[0], dtype=np.int64)
    col = np.asarray(edge_index[1], dtype=np.int64)
    ew = np.asarray(edge_weight, dtype=np.float64)

    deg = np.bincount(col, weights=ew, minlength=N_NODES) + 1.0
    dinv = 1.0 / np.sqrt(deg)

    loops = np.arange(N_NODES, dtype=np.int64)
    rows_all = np.concatenate([row, loops])
    cols_all = np.concatenate([col, loops])
    norm_all = np.concatenate([dinv[row] * ew * dinv[col], dinv * dinv]).astype(
        np.float32
    )

    tile_id = cols_all >> 7  # // 128
    order = np.argsort(tile_id, kind="stable")
    tile_sorted = tile_id[order]
    cnt = np.bincount(tile_sorted, minlength=NT_GLOBAL)
    maxc = int(np.max((cnt + P - 1) // P))

    starts = np.zeros(NT_GLOBAL, dtype=np.int64)
    starts[1:] = np.cumsum(cnt)[:-1]
    pos_in_tile = np.arange(tile_sorted.size) - starts[tile_sorted]
    slots = tile_sorted * (maxc * P) + pos_in_tile

    tot = NT_GLOBAL * maxc * P
    idx_flat = np.zeros(tot, dtype=np.int32)
    colrel_flat = np.zeros(tot, dtype=np.float32)
    norm_flat = np.zeros(tot, dtype=np.float32)
    idx_flat[slots] = rows_all[order].astype(np.int32)
    colrel_flat[slots] = (cols_all[order] & 127).astype(np.float32)
    norm_flat[slots] = norm_all[order]

    nch = TILES_PER_CORE * maxc
    idx_core = idx_flat.reshape(NCORES, nch, P)
    colrel_core = colrel_flat.reshape(NCORES, nch, P)
    norm_core = norm_flat.reshape(NCORES, nch, P)

    idx_maps = []
    cn_maps = []
    for c in range(NCORES):
        idx_maps.append(np.ascontiguousarray(idx_core[c].T))        # [P, nch]
        cn_maps.append(
            np.ascontiguousarray(
                np.concatenate([colrel_core[c].T, norm_core[c].T], axis=1)
            )
        )  # [P, 2*nch]
    return idx_maps, cn_maps, maxc


def _build_program(maxc):
    """Build + compile the SPMD Bass program (identical on all 8 cores)."""
    nch = TILES_PER_CORE * maxc
    dt = mybir.dt

    nc = bacc.Bacc("TRN2", target_bir_lowering=False, debug=False,
                   num_devices=NCORES)

    x_d = nc.declare_dram_parameter("x", [N_NODES, F_DIM], dt.float32,
                                    isOutput=False)
    idx_d = nc.declare_dram_parameter("idx", [P, nch], dt.int32, isOutput=False)
    cn_d = nc.declare_dram_parameter("cn", [P, 2 * nch], dt.float32,
                                     isOutput=False)
    w0t_d = nc.declare_dram_parameter("w0t", [F_DIM, F_DIM], dt.float32,
                                      isOutput=False)
    wiht_d = nc.declare_dram_parameter("wiht", [F_DIM, 3 * F_DIM], dt.float32,
                                       isOutput=False)
    whht_d = nc.declare_dram_parameter("whht", [F_DIM, 3 * F_DIM], dt.float32,
                                       isOutput=False)
    bias4_d = nc.declare_dram_parameter("bias4", [F_DIM, 4], dt.float32,
                                        isOutput=False)
    wlint_d = nc.declare_dram_parameter("wlint", [F_DIM, N_TARGET], dt.float32,
                                        isOutput=False)
    blin_d = nc.declare_dram_parameter("blin", [P, N_TARGET], dt.float32,
                                       isOutput=False)
    out_d = nc.declare_dram_parameter("out", [ROWS_PER_CORE, N_TARGET],
                                      dt.float32, isOutput=True)

    with tile.TileContext(nc) as tc:
        with (
            tc.tile_pool(name="const", bufs=1) as cpool,
            tc.tile_pool(name="work", bufs=8) as wpool,
            tc.tile_pool(name="epi", bufs=3) as epool,
            tc.tile_pool(name="psum", bufs=2, space="PSUM") as ppool,
        ):
            # ---- constants ----
            iota_i = cpool.tile([P, P], dtype=dt.int32)
            nc.gpsimd.iota(iota_i[:], pattern=[[1, P]], base=0,
                           channel_multiplier=0)
            iota_f = cpool.tile([P, P], dtype=dt.float32)
            nc.vector.tensor_copy(iota_f[:], iota_i[:])
            ident = cpool.tile([P, P], dtype=dt.float32)
            make_identity(nc, ident[:])

            idx_sb = cpool.tile([P, nch], dtype=dt.int32)
            cn_sb = cpool.tile([P, 2 * nch], dtype=dt.float32)
            nc.sync.dma_start(out=idx_sb[:], in_=idx_d[:])
            nc.sync.dma_start(out=cn_sb[:], in_=cn_d[:])

            w0t_sb = cpool.tile([P, F_DIM], dtype=dt.float32)
            wiht_sb = cpool.tile([P, 3 * F_DIM], dtype=dt.float32)
            whht_sb = cpool.tile([P, 3 * F_DIM], dtype=dt.float32)
            bias4_sb = cpool.tile([P, 4], dtype=dt.float32)
            wlint_sb = cpool.tile([P, N_TARGET], dtype=dt.float32)
            blin_sb = cpool.tile([P, N_TARGET], dtype=dt.float32)
            nc.sync.dma_start(out=w0t_sb[:], in_=w0t_d[:])
            nc.sync.dma_start(out=wiht_sb[:], in_=wiht_d[:])
            nc.sync.dma_start(out=whht_sb[:], in_=whht_d[:])
            nc.sync.dma_start(out=bias4_sb[:], in_=bias4_d[:])
            nc.sync.dma_start(out=wlint_sb[:], in_=wlint_d[:])
            nc.sync.dma_start(out=blin_sb[:], in_=blin_d[:])

            # ---- GRU weight evolution (transposed gates: [j, k]) ----
            sig = mybir.ActivationFunctionType.Sigmoid
            tanh = mybir.ActivationFunctionType.Tanh
            ident_f = mybir.ActivationFunctionType.Identity

            ps_r = ppool.tile([P, P], dtype=dt.float32, space="PSUM", tag="hpre")
            nc.tensor.matmul(out=ps_r[:], lhsT=wiht_sb[:, 0:128],
                             rhs=w0t_sb[:], start=True, stop=False)
            nc.tensor.matmul(out=ps_r[:], lhsT=whht_sb[:, 0:128],
                             rhs=w0t_sb[:], start=False, stop=True)
            rT = cpool.tile([P, P], dtype=dt.float32, tag="gru_rT")
            nc.scalar.activation(rT[:], ps_r[:], sig, bias=bias4_sb[:, 0:1])

            ps_z = ppool.tile([P, P], dtype=dt.float32, space="PSUM", tag="hpre")
            nc.tensor.matmul(out=ps_z[:], lhsT=wiht_sb[:, 128:256],
                             rhs=w0t_sb[:], start=True, stop=False)
            nc.tensor.matmul(out=ps_z[:], lhsT=whht_sb[:, 128:256],
                             rhs=w0t_sb[:], start=False, stop=True)
            zT = cpool.tile([P, P], dtype=dt.float32, tag="gru_zT")
            nc.scalar.activation(zT[:], ps_z[:], sig, bias=bias4_sb[:, 1:2])

            ps_in = ppool.tile([P, P], dtype=dt.float32, space="PSUM", tag="hpre")
            nc.tensor.matmul(out=ps_in[:], lhsT=wiht_sb[:, 256:384],
                             rhs=w0t_sb[:], start=True, stop=True)
            ps_hn = ppool.tile([P, P], dtype=dt.float32, space="PSUM", tag="hpre")
            nc.tensor.matmul(out=ps_hn[:], lhsT=whht_sb[:, 256:384],
                             rhs=w0t_sb[:], start=True, stop=True)
            hnT = cpool.tile([P, P], dtype=dt.float32, tag="gru_hnT")
            nc.scalar.activation(hnT[:], ps_hn[:], ident_f,
                                 bias=bias4_sb[:, 3:4])
            t1 = cpool.tile([P, P], dtype=dt.float32, tag="gru_t1")
            nc.vector.tensor_tensor(out=t1[:], in0=rT[:], in1=hnT[:],
                                    op=mybir.AluOpType.mult)
            nc.vector.tensor_tensor(out=t1[:], in0=t1[:], in1=ps_in[:],
                                    op=mybir.AluOpType.add)
            nT = cpool.tile([P, P], dtype=dt.float32, tag="gru_nT")
            nc.scalar.activation(nT[:], t1[:], tanh, bias=bias4_sb[:, 2:3])
            # W^T = n^T + z^T * (W0^T - n^T)
            t3 = cpool.tile([P, P], dtype=dt.float32, tag="gru_t3")
            nc.vector.tensor_tensor(out=t3[:], in0=w0t_sb[:], in1=nT[:],
                                    op=mybir.AluOpType.subtract)
            nc.vector.tensor_tensor(out=t3[:], in0=zT[:], in1=t3[:],
                                    op=mybir.AluOpType.mult)
            wT_sb = cpool.tile([P, P], dtype=dt.float32)
            nc.vector.tensor_tensor(out=wT_sb[:], in0=nT[:], in1=t3[:],
                                    op=mybir.AluOpType.add)
            # W [k, j] = transpose(W^T)
            ps_w = ppool.tile([P, P], dtype=dt.float32, space="PSUM", tag="hpre")
            nc.tensor.transpose(out=ps_w[:], in_=wT_sb[:], identity=ident[:])
            w_sb = cpool.tile([P, P], dtype=dt.float32)
            nc.scalar.copy(w_sb[:], ps_w[:])

            # ---- main loop over dest tiles ----
            for t in range(TILES_PER_CORE):
                hpreT_ps = ppool.tile([P, P], dtype=dt.float32, space="PSUM",
                                      tag="hpre")
                for ci in range(maxc):
                    j = t * maxc + ci
                    xg = wpool.tile([P, F_DIM], dtype=dt.float32, tag="xg")
                    nc.gpsimd.indirect_dma_start(
                        out=xg[:], out_offset=None, in_=x_d[:],
                        in_offset=bass.IndirectOffsetOnAxis(
                            ap=idx_sb[:, j:j + 1], axis=0),
                    )
                    a_mat = wpool.tile([P, P], dtype=dt.float32, tag="a")
                    nc.vector.scalar_tensor_tensor(
                        out=a_mat[:], in0=iota_f[:],
                        scalar=cn_sb[:, j:j + 1],
                        in1=cn_sb[:, nch + j:nch + j + 1].to_broadcast([P, P]),
                        op0=mybir.AluOpType.is_equal,
                        op1=mybir.AluOpType.mult,
                    )
                    nc.tensor.matmul(out=hpreT_ps[:], lhsT=xg[:], rhs=a_mat[:],
                                     start=(ci == 0), stop=(ci == maxc - 1))
                # epilogue: h^T = W^T @ hpre^T ; relu; head matmul; +bias
                hpreT_sb = epool.tile([P, P], dtype=dt.float32, tag="hpre_sb")
                nc.scalar.copy(hpreT_sb[:], hpreT_ps[:])
                hT_ps = ppool.tile([P, P], dtype=dt.float32, space="PSUM",
                                    tag="ht")
                nc.tensor.matmul(out=hT_ps[:], lhsT=w_sb[:], rhs=hpreT_sb[:],
                                 start=True, stop=True)
                hT_relu = epool.tile([P, P], dtype=dt.float32, tag="ht_sb")
                nc.scalar.activation(hT_relu[:], hT_ps[:],
                                     mybir.ActivationFunctionType.Relu)
                out_ps = ppool.tile([P, N_TARGET], dtype=dt.float32,
                                     space="PSUM", tag="out")
                nc.tensor.matmul(out=out_ps[:], lhsT=hT_relu[:],
                                 rhs=wlint_sb[:], start=True, stop=True)
                out_sb = epool.tile([P, N_TARGET], dtype=dt.float32,
                                    tag="out_sb")
                nc.vector.tensor_tensor(out=out_sb[:], in0=out_ps[:],
                                        in1=blin_sb[:],
                                        op=mybir.AluOpType.add)
                nc.sync.dma_start(out=out_d[t * P:(t + 1) * P, :],
                                  in_=out_sb[:])

    nc.compile()
    return nc


def kernel(x, edge_index, edge_weight, W0, Wih, Whh, bih, bhh, Wlin, blin):
    x = np.ascontiguousarray(np.asarray(x, dtype=np.float32))
    idx_maps, cn_maps, maxc = _host_prep(x, edge_index, edge_weight)

    if maxc not in _COMPILED:
        _COMPILED[maxc] = _build_program(maxc)
    nc = _COMPILED[maxc]

    W0 = np.asarray(W0, dtype=np.float32)
    Wih = np.asarray(Wih, dtype=np.float32)
    Whh = np.asarray(Whh, dtype=np.float32)
    bih = np.asarray(bih, dtype=np.float32)
    bhh = np.asarray(bhh, dtype=np.float32)
    Wlin = np.asarray(Wlin, dtype=np.float32)
    blin = np.asarray(blin, dtype=np.float32)

    w0t = np.ascontiguousarray(W0.T)
    wiht = np.ascontiguousarray(Wih.T)   # [F, 3F]
    whht = np.ascontiguousarray(Whh.T)
    bias4 = np.stack(
        [bih[0:128] + bhh[0:128], bih[128:256] + bhh[128:256],
         bih[256:384], bhh[256:384]], axis=1,
    ).astype(np.float32)                  # [128, 4]
    wlint = np.ascontiguousarray(Wlin.T)  # [F, 8]
    blin_rep = np.ascontiguousarray(np.tile(blin[None, :], (P, 1)))

    in_maps = []
    for c in range(NCORES):
        in_maps.append({
            "x": x, "idx": idx_maps[c], "cn": cn_maps[c],
            "w0t": w0t, "wiht": wiht, "whht": whht, "bias4": bias4,
            "wlint": wlint, "blin": blin_rep,
        })

    trace = os.environ.get("GCN_TRACE", "0") == "1"
    res = run_bass_kernel_spmd(
        nc, in_maps, list(range(NCORES)), trace=trace,
        trace_cores=list(range(NCORES)) if trace else None,
    )
    if trace and res.exec_time_ns is not None:
        print(f"HW exec time: {res.exec_time_ns} ns")

    parts = []
    for c in range(NCORES):
        rows = min(ROWS_PER_CORE, N_NODES - c * ROWS_PER_CORE)
        parts.append(res.results[c]["out"][:rows])
    return np.concatenate(parts, axis=0)
